# revision 1
# baseline (speedup 1.0000x reference)
# CMPN encoder Bass kernel for 8-core TRN2 (SPMD, molecule-sharded).
#
# Sharding: atoms/bonds row-sharded 8 ways (6400 atoms / 12800 bonds per core,
# aligned to molecule boundaries: core k owns mols [128k, 128k+128)).
# bf16 message tables replicated via ncfw AllGather each iteration; gathers are
# 128-row indirect DMAs. Row remap: atom a -> a-1 (a>=1), atom 0 -> row 51200
# (computed redundantly on every core). Bond b -> b-1, bond 0 -> row 102400.
import numpy as np
import ml_dtypes  # noqa
import concourse.bass as bass
import concourse.bacc as bacc
import concourse.tile as tile
from concourse import mybir
from concourse.masks import make_identity

F32 = mybir.dt.float32
BF16 = mybir.dt.bfloat16
I32 = mybir.dt.int32
AF = mybir.ActivationFunctionType
ALU = mybir.AluOpType

H = 300
B = 1024
A = 50
NA = 51201
NB = 102401
AFD = 133
BFD = 147
N_CORES = 8
NMOL = B // N_CORES            # 128 molecules per core
ATOMS = NMOL * A               # 6400 atoms per core
BONDS = 2 * ATOMS              # 12800 bonds per core
AT = ATOMS // 128              # 50 atom tiles
BT = BONDS // 128              # 100 bond tiles
MA_ROWS = NA - 1 + 128         # row 51200 = atom 0
MB_ROWS = NB - 1 + 128         # row 102400 = bond 0
DEPTH = 4

KCH = [(0, 128), (128, 256), (256, 300)]  # K chunks for H=300
NCH_MA = 1
NCH_MB = 1


WEIGHT_SPECS = (
    [("Wia", (AFD, H), BF16), ("Wib", (BFD, H), BF16)]
    + [(f"Wh{d}", (H, H), BF16) for d in range(DEPTH - 1)]
    + [(f"Wlr{p}", (H, H), BF16) for p in range(3)]
    + [("grub", (1, H), BF16)]
    + [(f"wihT{d}", (H, 3 * H), BF16) for d in "fb"]
    + [(f"whhT{d}", (H, 3 * H), BF16) for d in "fb"]
    + [(f"bih{d}", (1, 3 * H), BF16) for d in "fb"]
    + [(f"bhh{d}", (1, 3 * H), BF16) for d in "fb"]
    + [("Wo", (6 * 128, H), BF16), ("boT", (H, 1), F32)]
)


def prep_weights(inp):
    """Host-side transform of weight arrays -> dict matching WEIGHT_SPECS."""
    w = {k: np.asarray(inp[k], np.float32) for k in
         ["W_i_atom", "W_i_bond", "W_h", "W_lr", "gru_bias", "w_ih_f", "w_hh_f",
          "b_ih_f", "b_hh_f", "w_ih_b", "w_hh_b", "b_ih_b", "b_hh_b", "W_o", "b_o"]}
    out = {"Wia": w["W_i_atom"], "Wib": w["W_i_bond"]}
    for d in range(DEPTH - 1):
        out[f"Wh{d}"] = w["W_h"][d]
    for p in range(3):
        out[f"Wlr{p}"] = w["W_lr"][p * H:(p + 1) * H]
    out["grub"] = w["gru_bias"][None, :]
    for d in "fb":
        out[f"wihT{d}"] = w[f"w_ih_{d}"].T.copy()
        out[f"whhT{d}"] = w[f"w_hh_{d}"].T.copy()
        out[f"bih{d}"] = w[f"b_ih_{d}"][None, :]
        out[f"bhh{d}"] = w[f"b_hh_{d}"][None, :]
    Wo_pad = np.zeros((6 * 128, H), np.float32)
    for i, (k0, k1) in enumerate(KCH):
        Wo_pad[i * 128:i * 128 + (k1 - k0)] = w["W_o"][k0:k1]
        Wo_pad[(3 + i) * 128:(3 + i) * 128 + (k1 - k0)] = w["W_o"][H + k0:H + k1]
    out["Wo"] = Wo_pad
    out["boT"] = w["b_o"][:, None]
    res = {}
    for name, shape, dt in WEIGHT_SPECS:
        arr = np.ascontiguousarray(out[name])
        assert arr.shape == tuple(shape), (name, arr.shape, shape)
        res[name] = arr.astype(ml_dtypes.bfloat16 if dt == BF16 else np.float32)
    return res


def build_kernel(mp_iters=DEPTH - 1, do_final=True):
    nc = bacc.Bacc("TRN2", target_bir_lowering=False, debug=False,
                   num_devices=N_CORES)

    # ---- I/O -------------------------------------------------------------
    fa = nc.dram_tensor("fa", [(AT + 1) * 128, AFD], BF16, kind="ExternalInput")
    fb = nc.dram_tensor("fb", [(BT + 1) * 128, BFD], BF16, kind="ExternalInput")
    idx_a2b_A = nc.dram_tensor("idx_a2b_a", [128, (AT + 1) * 6], I32, kind="ExternalInput")
    idx_a2b_F = nc.dram_tensor("idx_a2b_f", [128, AT * 6], I32, kind="ExternalInput")
    idx_b2a = nc.dram_tensor("idx_b2a", [128, BT + 1], I32, kind="ExternalInput")
    idx_b2revb = nc.dram_tensor("idx_b2revb", [128, BT + 1], I32, kind="ExternalInput")
    mv_t = nc.dram_tensor("mv_t", [H, NMOL], F32, kind="ExternalOutput")

    # ---- internal DRAM ---------------------------------------------------
    MA = nc.dram_tensor("ma_table", [MA_ROWS, H], BF16, addr_space="Shared")
    MB = nc.dram_tensor("mb_table", [MB_ROWS, H], BF16, addr_space="Shared")
    ag_ma = nc.dram_tensor("ag_ma", [ATOMS, H], BF16)
    ag_mb = nc.dram_tensor("ag_mb", [BONDS, H], BF16)
    ma_sh = nc.dram_tensor("ma_sh", [(AT + 1) * 128, H], F32)
    ia_sh = nc.dram_tensor("ia_sh", [(AT + 1) * 128, H], BF16)
    ib_sh = nc.dram_tensor("ib_sh", [(BT + 1) * 128, H], F32)
    msg_d = nc.dram_tensor("msg_d", [A, NMOL, H], BF16)
    gi_d = {d: nc.dram_tensor(f"gi_{d}", [A, NMOL, 3 * H], F32) for d in "fb"}
    hT_d = {d: nc.dram_tensor(f"hT_{d}", [A, 3, 128, NMOL], BF16) for d in "fb"}

    rg = [list(range(N_CORES))]
    bf = lambda x: np.ascontiguousarray(x).astype(ml_dtypes.bfloat16)

    # weight tensors as per-core external inputs (host pre-transforms them)
    wdr = {}
    for name, shape, dt in WEIGHT_SPECS:
        wdr[name] = nc.dram_tensor(name, list(shape), dt, kind="ExternalInput")

    with tile.TileContext(nc) as tc:
        with tc.tile_pool(name="const", bufs=1) as cp:
            # ---- persistent constants -----------------------------------
            def load_const(name, arr_or_shape=None, dtype=BF16):
                dr = wdr[name]
                t = cp.tile(list(dr.shape), dtype, tag=name)
                nc.sync.dma_start(out=t[:], in_=dr[:])
                return t

            def wchunks(name, mat=None, dtype=BF16):
                dr = wdr[name]
                K = dr.shape[0]
                outs = []
                for i, k0 in enumerate(range(0, K, 128)):
                    k1 = min(k0 + 128, K)
                    t = cp.tile([k1 - k0, dr.shape[1]], dtype, tag=f"{name}_{i}")
                    nc.sync.dma_start(out=t[:], in_=dr[k0:k1, :])
                    outs.append(t)
                return outs

            ident = cp.tile([128, 128], F32, tag="ident")
            make_identity(nc, ident[:])
            identb = cp.tile([128, 128], BF16, tag="identb")
            nc.vector.tensor_copy(out=identb[:], in_=ident[:])
            ones = cp.tile([1, 128], BF16, tag="ones")
            nc.vector.memset(ones[:], 1.0)

            idxA = cp.tile([128, (AT + 1) * 6], I32, tag="idxA")
            nc.sync.dma_start(out=idxA[:], in_=idx_a2b_A[:])
            idxF = cp.tile([128, AT * 6], I32, tag="idxF")
            nc.sync.dma_start(out=idxF[:], in_=idx_a2b_F[:])
            idxBA = cp.tile([128, BT + 1], I32, tag="idxBA")
            nc.sync.dma_start(out=idxBA[:], in_=idx_b2a[:])
            idxBR = cp.tile([128, BT + 1], I32, tag="idxBR")
            nc.sync.dma_start(out=idxBR[:], in_=idx_b2revb[:])

            Wia = wchunks("Wia")
            Wib = wchunks("Wib")
            Wh = [wchunks(f"Wh{d}") for d in range(DEPTH - 1)]
            Wlr = [wchunks(f"Wlr{p}") for p in range(3)]
            grub = load_const("grub")
            wihT = {d: wchunks(f"wihT{d}") for d in "fb"}
            whhT = {d: wchunks(f"whhT{d}") for d in "fb"}
            bih = {d: load_const(f"bih{d}") for d in "fb"}
            bhh = {d: load_const(f"bhh{d}") for d in "fb"}
            WoC = wchunks("Wo")
            boTc = wchunks("boT", dtype=F32)

            # ============ helpers ========================================
            def transpose_chunks(pool, psum, src, dtype=BF16, eng="act", tag="tx"):
                outs = []
                for ci, (k0, k1) in enumerate(KCH):
                    pt = psum.tile([128, 128], src.dtype, tag="tp")
                    idt = ident if src.dtype == F32 else identb
                    nc.tensor.transpose(out=pt[:k1 - k0, :], in_=src[:, k0:k1],
                                        identity=idt[:])
                    st = pool.tile([128, 128], dtype, tag=f"{tag}{ci}")
                    if eng == "act":
                        nc.scalar.copy(out=st[:k1 - k0, :], in_=pt[:k1 - k0, :])
                    else:
                        nc.vector.tensor_copy(out=st[:k1 - k0, :], in_=pt[:k1 - k0, :])
                    outs.append(st)
                return outs

            def mm_kchunks(pt, lhsT_tiles, rhs_tiles, start=True, stop=True,
                           n0=0, n1=H):
                for ci, (lt, rt) in enumerate(zip(lhsT_tiles, rhs_tiles)):
                    klen = min(lt.shape[0], rt.shape[0])
                    nc.tensor.matmul(out=pt[:], lhsT=lt[:klen, :],
                                     rhs=rt[:klen, n0:n1],
                                     start=(start and ci == 0),
                                     stop=(stop and ci == len(lhsT_tiles) - 1),
                                     skip_group_check=True)

            def agg_tile(pool, idx_tile, col0):
                G = pool.tile([128, 6 * H], BF16, tag="G")
                for j in range(6):
                    nc.gpsimd.indirect_dma_start(
                        out=G[:, j * H:(j + 1) * H], out_offset=None, in_=MB[:, :],
                        in_offset=bass.IndirectOffsetOnAxis(
                            ap=idx_tile[:, col0 + j:col0 + j + 1], axis=0))
                S = pool.tile([128, 3 * H], F32, tag="S")
                nc.vector.tensor_add(out=S[:], in0=G[:, :3 * H], in1=G[:, 3 * H:])
                M = pool.tile([128, 3 * H], BF16, tag="M")
                nc.vector.tensor_tensor(out=M[:], in0=G[:, :3 * H], in1=G[:, 3 * H:],
                                        op=ALU.max)
                s = pool.tile([128, H], F32, tag="sS")
                nc.vector.tensor_add(out=s[:], in0=S[:, 0:H], in1=S[:, H:2 * H])
                nc.vector.tensor_add(out=s[:], in0=s[:], in1=S[:, 2 * H:])
                m = pool.tile([128, H], F32, tag="mM")
                nc.vector.tensor_tensor(out=m[:], in0=M[:, 0:H], in1=M[:, H:2 * H],
                                        op=ALU.max)
                nc.vector.tensor_tensor(out=m[:], in0=m[:], in1=M[:, 2 * H:], op=ALU.max)
                agg = pool.tile([128, H], F32, tag="agg")
                nc.vector.tensor_mul(out=agg[:], in0=s[:], in1=m[:])
                return agg

            def input_proj(pool, psum, src_dram, t, fdim, Wch, out_f32_dram,
                           bf_dram, ag_dram, table, nt, table_row):
                x = pool.tile([128, fdim], BF16, tag="xin")
                nc.sync.dma_start(out=x[:], in_=src_dram[t * 128:(t + 1) * 128, :])
                lhs = []
                for ci, (k0, k1) in enumerate([(0, 128), (128, fdim)]):
                    pt = psum.tile([128, 128], BF16, tag="tp")
                    nc.tensor.transpose(out=pt[:k1 - k0, :], in_=x[:, k0:k1],
                                        identity=identb[:])
                    st = pool.tile([128, 128], BF16, tag=f"ptx{ci}")
                    nc.scalar.copy(out=st[:k1 - k0, :], in_=pt[:k1 - k0, :])
                    lhs.append((st, k1 - k0))
                pm = psum.tile([128, H], F32, tag="mm")
                for ci, ((st, klen), wt) in enumerate(zip(lhs, Wch)):
                    nc.tensor.matmul(out=pm[:], lhsT=st[:klen, :], rhs=wt[:klen, :],
                                     start=(ci == 0), stop=(ci == 1),
                                     skip_group_check=True)
                of = pool.tile([128, H], F32, tag="of")
                nc.scalar.activation(out=of[:], in_=pm[:], func=AF.Relu)
                if out_f32_dram is not None:
                    nc.sync.dma_start(out=out_f32_dram[t * 128:(t + 1) * 128, :], in_=of[:])
                ob = pool.tile([128, H], BF16, tag="ob")
                nc.vector.tensor_copy(out=ob[:], in_=of[:])
                if bf_dram is not None:
                    nc.sync.dma_start(out=bf_dram[t * 128:(t + 1) * 128, :], in_=ob[:])
                if t < nt:
                    nc.sync.dma_start(out=ag_dram[t * 128:(t + 1) * 128, :], in_=ob[:])
                else:
                    nc.sync.dma_start(out=table[table_row:table_row + 1, :], in_=ob[:1, :])

            # ============ stage 0 ========================================
            with (tc.tile_pool(name="s0", bufs=3) as pool,
                  tc.tile_pool(name="s0p", bufs=2, space="PSUM") as psum):
                for t in range(AT + 1):
                    input_proj(pool, psum, fa, t, AFD, Wia, ma_sh, ia_sh,
                               ag_ma, MA, AT, NA - 1)
                for t in range(BT + 1):
                    input_proj(pool, psum, fb, t, BFD, Wib, ib_sh, None,
                               ag_mb, MB, BT, NB - 1)

            def ag_chunks(is_ma):
                src = ag_ma if is_ma else ag_mb
                dst = MA if is_ma else MB
                rows = ATOMS if is_ma else BONDS
                nch = NCH_MA if is_ma else NCH_MB
                sz = rows // nch
                for cch in range(nch):
                    nc.gpsimd.collective_compute(
                        "AllGather", ALU.bypass, replica_groups=rg,
                        ins=[src[cch * sz:(cch + 1) * sz, :]],
                        outs=[dst[cch * sz * N_CORES:(cch + 1) * sz * N_CORES, :]])

            ag_chunks(True)
            ag_chunks(False)

            # ============ message passing ================================
            for d in range(mp_iters):
                with tc.tile_pool(name=f"A{d}", bufs=4) as pool:
                    for t in range(AT + 1):
                        agg = agg_tile(pool, idxA, t * 6)
                        mo = pool.tile([128, H], F32, tag="mo")
                        nc.sync.dma_start(out=mo[:], in_=ma_sh[t * 128:(t + 1) * 128, :])
                        nc.vector.tensor_add(out=mo[:], in0=mo[:], in1=agg[:])
                        nc.sync.dma_start(out=ma_sh[t * 128:(t + 1) * 128, :], in_=mo[:])
                        mb16 = pool.tile([128, H], BF16, tag="mb16")
                        nc.vector.tensor_copy(out=mb16[:], in_=mo[:])
                        if t < AT:
                            nc.sync.dma_start(out=ag_ma[t * 128:(t + 1) * 128, :],
                                              in_=mb16[:])
                        else:
                            nc.sync.dma_start(out=MA[NA - 1:NA, :], in_=mb16[:1, :])
                ag_chunks(True)

                with (tc.tile_pool(name=f"B{d}", bufs=4) as pool,
                      tc.tile_pool(name=f"B{d}p", bufs=2, space="PSUM") as psum):
                    for t in range(BT + 1):
                        g2 = pool.tile([128, H], BF16, tag="g2")
                        nc.gpsimd.indirect_dma_start(
                            out=g2[:], out_offset=None, in_=MB[:, :],
                            in_offset=bass.IndirectOffsetOnAxis(
                                ap=idxBR[:, t:t + 1], axis=0))
                        g1 = pool.tile([128, H], BF16, tag="g1")
                        nc.gpsimd.indirect_dma_start(
                            out=g1[:], out_offset=None, in_=MA[:, :],
                            in_offset=bass.IndirectOffsetOnAxis(
                                ap=idxBA[:, t:t + 1], axis=0))
                        sub = pool.tile([128, H], F32, tag="sub")
                        nc.vector.tensor_sub(out=sub[:], in0=g1[:], in1=g2[:])
                        lhs = transpose_chunks(pool, psum, sub)
                        pm = psum.tile([128, H], F32, tag="mm")
                        mm_kchunks(pm, lhs, Wh[d % (DEPTH - 1)])
                        ib = pool.tile([128, H], F32, tag="ib")
                        nc.sync.dma_start(out=ib[:], in_=ib_sh[t * 128:(t + 1) * 128, :])
                        pre = pool.tile([128, H], F32, tag="pre")
                        nc.vector.tensor_add(out=pre[:], in0=pm[:], in1=ib[:])
                        mbt = pool.tile([128, H], BF16, tag="mbt")
                        nc.scalar.activation(out=mbt[:], in_=pre[:], func=AF.Relu)
                        if t < BT:
                            nc.sync.dma_start(out=ag_mb[t * 128:(t + 1) * 128, :],
                                              in_=mbt[:])
                        else:
                            nc.sync.dma_start(out=MB[NB - 1:NB, :], in_=mbt[:1, :])
                ag_chunks(False)

            if do_final:
                with tc.tile_pool(name="h0p", bufs=1) as h0p:
                    h0 = h0p.tile([128, H], F32, tag="h0")
                    nc.vector.memset(h0[:], -1e30)

                    # ---- F2: readout + gi precompute (t-major) ----------
                    ma_r = ma_sh.ap()[0:ATOMS, :].rearrange("(m t) h -> m t h", t=A)
                    ia_r = ia_sh.ap()[0:ATOMS, :].rearrange("(m t) h -> m t h", t=A)
                    with (tc.tile_pool(name="F", bufs=3) as pool,
                          tc.tile_pool(name="Fp", bufs=2, space="PSUM") as psum,
                          tc.tile_pool(name="Fg", bufs=2, space="PSUM") as psg):
                        for t in range(A):
                            agg = agg_tile(pool, idxF, t * 6)
                            lhs_a = transpose_chunks(pool, psum, agg, tag="ta")
                            mo = pool.tile([128, H], F32, tag="mo")
                            nc.sync.dma_start(out=mo[:], in_=ma_r[:NMOL, t, :])
                            mo16 = pool.tile([128, H], BF16, tag="mo16")
                            nc.vector.tensor_copy(out=mo16[:], in_=mo[:])
                            lhs_m = transpose_chunks(pool, psum, mo16, tag="tm")
                            ia = pool.tile([128, H], BF16, tag="ia")
                            nc.sync.dma_start(out=ia[:], in_=ia_r[:NMOL, t, :])
                            lhs_i = transpose_chunks(pool, psum, ia, tag="ti")
                            pm = psum.tile([128, H], F32, tag="mm")
                            mm_kchunks(pm, lhs_a, Wlr[0], start=True, stop=False)
                            mm_kchunks(pm, lhs_m, Wlr[1], start=False, stop=False)
                            mm_kchunks(pm, lhs_i, Wlr[2], start=False, stop=True)
                            hid = pool.tile([128, H], F32, tag="hid")
                            nc.vector.tensor_copy(out=hid[:], in_=pm[:])
                            nc.vector.tensor_tensor(out=h0[:], in0=h0[:], in1=hid[:],
                                                    op=ALU.max)
                            nc.tensor.matmul(out=pm[:], lhsT=ones[:, :], rhs=grub[:, :],
                                             start=False, stop=True,
                                             skip_group_check=True)
                            msg = pool.tile([128, H], BF16, tag="msg")
                            nc.scalar.activation(out=msg[:], in_=pm[:], func=AF.Relu)
                            nc.sync.dma_start(out=msg_d[t, :, :], in_=msg[:])
                            lhs_x = transpose_chunks(pool, psum, msg, tag="txx")
                            for di in "fb":
                                for g in range(3):
                                    pg = psg.tile([128, H], F32, tag="gi")
                                    mm_kchunks(pg, lhs_x, wihT[di], start=True,
                                               stop=False, n0=g * H, n1=(g + 1) * H)
                                    nc.tensor.matmul(
                                        out=pg[:], lhsT=ones[:, :],
                                        rhs=bih[di][:, g * H:(g + 1) * H],
                                        start=False, stop=(g == 2),
                                        skip_group_check=True)
                                    if g < 2:
                                        nc.tensor.matmul(
                                            out=pg[:], lhsT=ones[:, :],
                                            rhs=bhh[di][:, g * H:(g + 1) * H],
                                            start=False, stop=True,
                                            skip_group_check=True)
                                    gt = pool.tile([128, H], F32, tag="gt")
                                    nc.scalar.copy(out=gt[:], in_=pg[:])
                                    nc.sync.dma_start(out=gi_d[di][t, :, g * H:(g + 1) * H], in_=gt[:])

                    # ---- GRU -------------------------------------------
                    with (tc.tile_pool(name="gruh", bufs=2) as hp,
                          tc.tile_pool(name="gru", bufs=3) as gp,
                          tc.tile_pool(name="grut", bufs=2) as gtp,
                          tc.tile_pool(name="grutp", bufs=2, space="PSUM") as ptp,
                          tc.tile_pool(name="grup", bufs=2, space="PSUM") as pgh):
                        hcur, hT = {}, {}
                        for di in "fb":
                            ht = hp.tile([128, H], F32, tag=f"h_{di}")
                            nc.vector.tensor_copy(out=ht[:], in_=h0[:])
                            hcur[di] = ht
                            hT[di] = transpose_chunks(gtp, ptp, ht, eng="dve",
                                                      tag=f"hx{di}")
                        for step in range(A):
                            for di in "fb":
                                t = step if di == "f" else A - 1 - step
                                h = hcur[di]
                                gh = []
                                for g in range(3):
                                    pg = pgh.tile([128, H], F32, tag=f"gh{g}")
                                    mm_kchunks(pg, hT[di], whhT[di], start=True,
                                               stop=(g < 2), n0=g * H, n1=(g + 1) * H)
                                    if g == 2:
                                        nc.tensor.matmul(
                                            out=pg[:], lhsT=ones[:, :],
                                            rhs=bhh[di][:, 2 * H:3 * H],
                                            start=False, stop=True,
                                            skip_group_check=True)
                                    gh.append(pg)
                                giw = gp.tile([128, 3 * H], F32, tag="gil")
                                nc.sync.dma_start(out=giw[:], in_=gi_d[di][t, :, :])
                                gi = [giw[:, g * H:(g + 1) * H] for g in range(3)]
                                rz = gp.tile([128, 2 * H], F32, tag="rz")
                                nc.vector.tensor_add(out=rz[:, 0:H], in0=gi[0], in1=gh[0][:])
                                nc.vector.tensor_add(out=rz[:, H:2 * H], in0=gi[1], in1=gh[1][:])
                                nc.scalar.activation(out=rz[:], in_=rz[:], func=AF.Sigmoid)
                                r = rz[:, 0:H]
                                z = rz[:, H:2 * H]
                                n_ = gp.tile([128, H], F32, tag="n")
                                nc.vector.tensor_mul(out=n_[:], in0=r, in1=gh[2][:])
                                nc.vector.tensor_add(out=n_[:], in0=n_[:], in1=gi[2])
                                nc.scalar.activation(out=n_[:], in_=n_[:], func=AF.Tanh)
                                hn = hp.tile([128, H], F32, tag=f"h_{di}")
                                nc.vector.tensor_sub(out=hn[:], in0=h[:], in1=n_[:])
                                nc.vector.tensor_mul(out=hn[:], in0=z, in1=hn[:])
                                nc.vector.tensor_add(out=hn[:], in0=hn[:], in1=n_[:])
                                hcur[di] = hn
                                hT[di] = transpose_chunks(gtp, ptp, hn, eng="dve",
                                                          tag=f"hx{di}")
                                for ci, (k0, k1) in enumerate(KCH):
                                    nc.sync.dma_start(
                                        out=hT_d[di][t, ci, 0:k1 - k0, :],
                                        in_=hT[di][ci][:k1 - k0, :])

                    # ---- F4: output projection + mol mean ---------------
                    with (tc.tile_pool(name="F4", bufs=3) as pool4,
                          tc.tile_pool(name="F4a", bufs=1) as accp,
                          tc.tile_pool(name="F4p", bufs=2, space="PSUM") as psum4):
                        boT_sb, mvacc = [], []
                        for ci, (k0, k1) in enumerate(KCH):
                            mt = accp.tile([128, NMOL], F32, tag=f"mv{ci}")
                            nc.vector.memset(mt[:], 0.0)
                            mvacc.append(mt)
                        for t in range(A):
                            cat = []
                            for di in "fb":
                                for ci, (k0, k1) in enumerate(KCH):
                                    ct = pool4.tile([128, NMOL], BF16, tag=f"c{di}{ci}")
                                    nc.sync.dma_start(
                                        out=ct[:k1 - k0, :],
                                        in_=hT_d[di][t, ci, 0:k1 - k0, :])
                                    cat.append((ct, k1 - k0))
                            for mc, (m0, m1) in enumerate(KCH):
                                pm = psum4.tile([128, NMOL], F32, tag="mm")
                                for ii, (ct, klen) in enumerate(cat):
                                    nc.tensor.matmul(
                                        out=pm[:m1 - m0, :],
                                        lhsT=WoC[ii][:klen, m0:m1],
                                        rhs=ct[:klen, :],
                                        start=(ii == 0), stop=(ii == 5),
                                        skip_group_check=True)
                                ah = pool4.tile([128, NMOL], F32, tag="ah")
                                nc.scalar.activation(out=ah[:m1 - m0, :],
                                                     in_=pm[:m1 - m0, :], func=AF.Relu,
                                                     bias=boTc[mc][:m1 - m0, :])
                                nc.vector.tensor_add(out=mvacc[mc][:m1 - m0, :],
                                                     in0=mvacc[mc][:m1 - m0, :],
                                                     in1=ah[:m1 - m0, :])
                        for mc, (m0, m1) in enumerate(KCH):
                            nc.vector.tensor_scalar_mul(mvacc[mc][:m1 - m0, :],
                                                        mvacc[mc][:m1 - m0, :], 1.0 / A)
                            nc.sync.dma_start(out=mv_t[m0:m1, :],
                                              in_=mvacc[mc][:m1 - m0, :])

    nc.compile()
    return nc


# =====================  host-side prep  ==================================

def _remap_chunk_major(x, per_core, nch):
    """Global id (id 0 = special last row) -> chunk-major table row:
    row = c*8*sz + k*sz + l%sz with k = owner core, l = local row, c = l//sz."""
    x = np.asarray(x, np.int64)
    sz = per_core // nch
    k = (x - 1) // per_core
    l = (x - 1) % per_core
    c = l // sz
    row = c * N_CORES * sz + k * sz + (l % sz)
    return np.where(x >= 1, row, per_core * N_CORES).astype(np.int32)


def remap_atom(a):
    return _remap_chunk_major(a, ATOMS, NCH_MA)


def remap_bond(b):
    return _remap_chunk_major(b, BONDS, NCH_MB)


def prep_inputs(inp):
    f_atoms = np.asarray(inp["f_atoms"], np.float32)
    f_bonds = np.asarray(inp["f_bonds"], np.float32)
    a2b_r = remap_bond(np.asarray(inp["a2b"]))
    b2a_r = remap_atom(np.asarray(inp["b2a"]))
    b2revb_r = remap_bond(np.asarray(inp["b2revb"]))

    ins = []
    for k in range(N_CORES):
        atom_ids = np.concatenate([np.arange(6400 * k + 1, 6400 * k + 6401),
                                   np.zeros(128, np.int64)])
        bond_ids = np.concatenate([np.arange(12800 * k + 1, 12800 * k + 12801),
                                   np.zeros(128, np.int64)])
        fa = f_atoms[atom_ids].astype(ml_dtypes.bfloat16)
        fb = f_bonds[bond_ids].astype(ml_dtypes.bfloat16)
        idxA = np.zeros((128, (AT + 1) * 6), np.int32)
        for t in range(AT + 1):
            idxA[:, t * 6:(t + 1) * 6] = a2b_r[atom_ids[t * 128:(t + 1) * 128]]
        idxF = np.zeros((128, AT * 6), np.int32)
        for t in range(A):
            rows = 6400 * k + 1 + np.arange(NMOL) * A + t
            idxF[:, t * 6:(t + 1) * 6] = a2b_r[rows]
        idxBA = np.zeros((128, BT + 1), np.int32)
        idxBR = np.zeros((128, BT + 1), np.int32)
        for t in range(BT + 1):
            rows = bond_ids[t * 128:(t + 1) * 128]
            idxBA[:, t] = b2a_r[rows]
            idxBR[:, t] = b2revb_r[rows]
        ins.append({"fa": fa, "fb": fb, "idx_a2b_a": idxA, "idx_a2b_f": idxF,
                    "idx_b2a": idxBA, "idx_b2revb": idxBR})
    return ins


def assemble_output(results):
    return np.concatenate([np.ascontiguousarray(r["mv_t"]).T for r in results], axis=0)


_CACHE = {}


def kernel(**inputs) -> np.ndarray:
    """Full-input entry point: shards across 8 NeuronCores, runs the bass
    kernel via run_bass_kernel_spmd, gathers the full [1024, 300] output."""
    from concourse.bass_utils import run_bass_kernel_spmd
    if "nc" not in _CACHE:
        _CACHE["nc"] = build_kernel()
    nc = _CACHE["nc"]
    wmap = prep_weights(inputs)
    ins = prep_inputs(inputs)
    for m in ins:
        m.update(wmap)
    res = run_bass_kernel_spmd(nc, ins, core_ids=list(range(N_CORES)))
    return assemble_output(res.results).astype(np.float32)

